# revision 5
# baseline (speedup 1.0000x reference)
"""Trainium2 Bass kernel for banded-cosine-similarity QA span logits.

Contract: kernel(**inputs) takes FULL inputs (sequence_outputs [8,2048,2048] f32,
idxs [8,2] int64) and returns the full output tuple (start_logits, end_logits),
each [8,2048] f32.  Sharding: pure data parallel, one example per NeuronCore.

Transfer format: seq is shipped as int16 (round(x*32767/absmax)).  Cosine
similarity is invariant to a uniform scale on seq (numerator and denominator
are both linear in it), so the integer values are used directly on device with
no dequant — only the q-row vectors (rows 1 and sep0-1) are shipped exact f32.
This halves host->device bytes, which dominates wall time over the axon tunnel,
at ~1e-5 output error (vs 2e-2 budget).

Per-core computation (S=2048 rows, H=2048 hidden, band W=30):
  dot1 = seq @ q1, dot2 = seq @ q2, nsq = rowsum(seq^2)   (the memory-bound part)
  sim[i,w] = (dot1[i]+dot2[i+w]) / (qnorm*sqrt(nsq[i]+nsq[i+w]))  masked band
  start = rowmax, end = anti-diagonal scatter-max of the row-argmax, plus a
  mean/std sign-flip heuristic.

Engine split for the heavy reductions over the [2048,2048] int16 matrix:
  - ScalarE (ACT): nsq via activation(Square, accum_out) straight off int16
  - VectorE (DVE): dot1/dot2 via scalar_tensor_tensor int16*f32 with accum
Band masks and row-valid are built on device from iota + sep bounds, so the
only inputs are seq [S,H] int16 and aux [128,36] f32 (q chunks + sep bounds).
"""

import numpy as np
from contextlib import ExitStack

import concourse.bass as bass
import concourse.tile as tile
import concourse.bacc as bacc
from concourse import mybir
from concourse.bass_utils import run_bass_kernel_spmd

f32 = mybir.dt.float32
i16 = mybir.dt.int16
i32 = mybir.dt.int32
AF = mybir.ActivationFunctionType
OP = mybir.AluOpType

B = 8
S = 2048
H = 2048
W = 30
P = 128
T = S // P          # 16 row tiles
C = H // P          # 16 h chunks
NEG = -1.0e30
AUXW = 2 * C + 4    # qb cols + sep0+1, sep1, pad


def _emit(tc, ctx, aps):
    nc = tc.nc
    seq_d = aps["seq"]
    aux_d = aps["aux"]
    out_d = aps["out"]
    qfd = aps["qfd"]
    d2f = aps["d2f"]
    sc_d = aps["sc"]
    scb_d = aps["scb"]
    nsf = aps["nsf"]

    persist = ctx.enter_context(tc.tile_pool(name="persist", bufs=1))
    xpool = ctx.enter_context(tc.tile_pool(name="xpool", bufs=3))
    scr_act_p = ctx.enter_context(tc.tile_pool(name="scr_act", bufs=2))
    scr_dve_p = ctx.enter_context(tc.tile_pool(name="scr_dve", bufs=2))
    pst_p = ctx.enter_context(tc.tile_pool(name="pst", bufs=2, space="PSUM"))
    psh_p = ctx.enter_context(tc.tile_pool(name="psh", bufs=4, space="PSUM"))

    # ---- constants / persistent tiles ----
    # bigI[k, y] = 1 iff y == k + W: slices give shifted identities
    bigI = persist.tile([P, P + 2 * W + P], f32)
    nc.gpsimd.memset(bigI[:], 0.0)
    nc.gpsimd.affine_select(
        out=bigI[:], in_=bigI[:], compare_op=OP.not_equal, fill=1.0,
        base=W, channel_multiplier=1, pattern=[[-1, P + 2 * W + P]])
    ones = persist.tile([P, 1], f32)
    nc.vector.memset(ones[:], 1.0)
    zeros16 = persist.tile([P, T], f32)
    nc.vector.memset(zeros16[:], 0.0)
    negm001 = persist.tile([P, T], f32)
    nc.vector.memset(negm001[:], -0.001)
    ninf_big = persist.tile([P, T * W], f32)
    nc.vector.memset(ninf_big[:], NEG)
    zpad = persist.tile([1, 32], f32)
    nc.vector.memset(zpad[:], 0.0)

    aux_sb = persist.tile([P, AUXW], f32)
    nc.sync.dma_start(aux_sb[:], aux_d[:])
    qb_sb = aux_sb[:, 0:2 * C]
    s0p1 = aux_sb[:, 2 * C:2 * C + 1]      # sep0 + 1, replicated per row
    s1c = aux_sb[:, 2 * C + 1:2 * C + 2]   # sep1

    # ---- q12b: [P, 2H] partition-broadcast of (q1, q2) in h-order ----
    # qb[p, 2c+j] = qj[128c + p]; bounce through DRAM to flatten to h-order,
    # then replicate across partitions by doubling SBUF->SBUF DMAs (HW DGE
    # mishandles wide 0-step partition broadcasts from DRAM).
    for j in range(2):
        nc.gpsimd.dma_start(
            bass.AP(qfd.tensor, j * H, [[1, P], [P, C]]),
            aux_sb[:, j:2 * C:2])
    q12b = persist.tile([P, 2 * H], f32)
    nc.gpsimd.dma_start(q12b[0:1, :], qfd[:].unsqueeze(0))
    k = 1
    while k < P:
        nc.gpsimd.dma_start(q12b[k:2 * k, :], q12b[0:k, :])
        k *= 2
    q1b = q12b[:, 0:H]
    q2b = q12b[:, H:2 * H]

    dot1_cols = persist.tile([P, T], f32)
    dot2_cols = persist.tile([P, T], f32)
    nsq_cols = persist.tile([P, T], f32)

    # ---- qnorm^2 (from exact f32 q chunks) ----
    qscr = persist.tile([P, 2 * C], f32)
    qcol = persist.tile([P, 1], f32)
    nc.scalar.activation(qscr[:], qb_sb, AF.Square, accum_out=qcol[:])
    ps_q = pst_p.tile([1, 1], f32, tag="ps_small")
    nc.tensor.matmul(ps_q[:], ones[:], qcol[:], start=True, stop=True)
    qn2_s = persist.tile([1, 1], f32)
    nc.vector.tensor_copy(qn2_s[:], ps_q[:])

    # SBUF partition-broadcast of a [1,1] scalar requires a DRAM bounce
    def bcast_scalar(s11, out_p1, slot):
        nc.sync.dma_start(sc_d[0:1, slot:slot + 1], s11[:])
        nc.sync.dma_start(out_p1[:], sc_d[0:1, slot:slot + 1].broadcast_to([P, 1]))

    qn2_b = persist.tile([P, 1], f32)
    bcast_scalar(qn2_s, qn2_b, 0)

    # ---- band masks from iota + sep bounds ----
    # row r = 128t + p; valid(r, w) = (r >= sep0+1) & (r < sep1) & (r+w < sep1)
    it_r = persist.tile([P, T], i32)
    nc.gpsimd.iota(it_r[:], pattern=[[P, T]], base=0, channel_multiplier=1)
    it_rw = persist.tile([P, T * W], i32)
    nc.gpsimd.iota(it_rw[:], pattern=[[P, T], [1, W]], base=0,
                   channel_multiplier=1)
    rf = persist.tile([P, T], f32)
    nc.vector.tensor_copy(rf[:], it_r[:])
    r3f = persist.tile([P, T * W], f32)
    nc.vector.tensor_copy(r3f[:], it_rw[:])
    rvf = persist.tile([P, T], f32)
    nc.vector.tensor_tensor(out=rvf[:], in0=rf[:],
                            in1=s0p1.broadcast_to([P, T]), op=OP.is_ge)
    rtmp = persist.tile([P, T], f32)
    nc.vector.tensor_tensor(out=rtmp[:], in0=rf[:],
                            in1=s1c.broadcast_to([P, T]), op=OP.is_lt)
    nc.vector.tensor_tensor(out=rvf[:], in0=rvf[:], in1=rtmp[:], op=OP.mult)
    m2 = persist.tile([P, T * W], f32)
    nc.vector.tensor_tensor(out=m2[:], in0=r3f[:],
                            in1=s1c.broadcast_to([P, T * W]), op=OP.is_lt)
    band = persist.tile([P, T * W], f32)
    nc.vector.tensor_tensor(
        out=band[:].rearrange("p (t w) -> p t w", w=W),
        in0=m2[:].rearrange("p (t w) -> p t w", w=W),
        in1=rvf[:].unsqueeze(2).broadcast_to([P, T, W]), op=OP.mult)
    mask_sb = persist.tile([P, T * W], f32)
    nc.vector.tensor_scalar(out=mask_sb[:], in0=band[:], scalar1=1.0e30,
                            scalar2=NEG, op0=OP.mult, op1=OP.add)

    # ---- phase A: per row-tile reductions straight off int16 ----
    for t in range(T):
        x = xpool.tile([P, H], i16, tag="x")
        eng = nc.sync if t % 2 == 0 else nc.scalar
        eng.dma_start(x[:], seq_d[t * P:(t + 1) * P, :])

        sa = scr_act_p.tile([P, H], f32, tag="sa")
        nc.scalar.activation(sa[:], x[:], AF.Square,
                             accum_out=nsq_cols[:, t:t + 1])
        sv = scr_dve_p.tile([P, H], f32, tag="sv")
        nc.vector.scalar_tensor_tensor(
            out=sv[:], in0=x[:], scalar=1.0, in1=q1b,
            op0=OP.mult, op1=OP.mult, accum_out=dot1_cols[:, t:t + 1])
        sv2 = scr_dve_p.tile([P, H], f32, tag="sv")
        nc.vector.scalar_tensor_tensor(
            out=sv2[:], in0=x[:], scalar=1.0, in1=q2b,
            op0=OP.mult, op1=OP.mult, accum_out=dot2_cols[:, t:t + 1])

    # ---- phase B: flatten vectors to DRAM, band-gather back ----
    d2flat_w = bass.AP(d2f.tensor, 0, [[1, P], [P, T]])
    nc.sync.dma_start(d2flat_w, dot2_cols[:])
    nsflat_w = bass.AP(nsf.tensor, 0, [[1, P], [P, T]])
    nc.sync.dma_start(nsflat_w, nsq_cols[:])
    nc.sync.dma_start(bass.AP(d2f.tensor, S, [[32, 1], [1, 32]]), zpad[:])
    nc.sync.dma_start(bass.AP(nsf.tensor, S, [[32, 1], [1, 32]]), zpad[:])

    d2_all = persist.tile([P, T * W], f32)
    nc.sync.dma_start(
        d2_all[:].rearrange("p (t w) -> p t w", w=W),
        bass.AP(d2f.tensor, 0, [[1, P], [P, T], [1, W]]))
    n2_all = persist.tile([P, T * W], f32)
    nc.sync.dma_start(
        n2_all[:].rearrange("p (t w) -> p t w", w=W),
        bass.AP(nsf.tensor, 0, [[1, P], [P, T], [1, W]]))

    # ---- phase C: banded similarity, max, scatter-max ----
    d1v = dot1_cols[:].unsqueeze(2).broadcast_to([P, T, W])
    nsv = nsq_cols[:].unsqueeze(2).broadcast_to([P, T, W])

    s_all = persist.tile([P, T * W], f32)
    nc.vector.tensor_tensor(out=s_all[:].rearrange("p (t w) -> p t w", w=W),
                            in0=n2_all[:].rearrange("p (t w) -> p t w", w=W),
                            in1=nsv, op=OP.add)
    # EPS-style clamp: rows outside [sep0+1, sep1) are shipped as zeros, so
    # masked (i,j) pairs can have nsq_i+nsq_j == 0; without the clamp that
    # makes sim = 0*inf = NaN which would poison the row max.  Real pairs
    # have s_all ~ 1e11, so 1.0 never clamps them.
    nc.vector.tensor_scalar_max(s_all[:], s_all[:], 1.0)
    den = persist.tile([P, T * W], f32)
    nc.scalar.activation(den[:], s_all[:], AF.Sqrt, scale=qn2_b[:])
    num = persist.tile([P, T * W], f32)
    nc.vector.tensor_tensor(out=num[:].rearrange("p (t w) -> p t w", w=W),
                            in0=d2_all[:].rearrange("p (t w) -> p t w", w=W),
                            in1=d1v, op=OP.add)
    rden = persist.tile([P, T * W], f32)
    nc.vector.reciprocal(rden[:], den[:])
    simv = persist.tile([P, T * W], f32)
    nc.vector.tensor_tensor(out=simv[:], in0=num[:], in1=rden[:], op=OP.mult)
    simm = persist.tile([P, T * W], f32)
    nc.vector.tensor_tensor(out=simm[:], in0=simv[:], in1=mask_sb[:], op=OP.add)

    smax = persist.tile([P, T], f32)
    nc.vector.tensor_reduce(smax[:], simm[:].rearrange("p (t w) -> p t w", w=W),
                            axis=mybir.AxisListType.X, op=OP.max)

    eq = persist.tile([P, T * W], mybir.dt.uint8)
    nc.vector.tensor_tensor(out=eq[:].rearrange("p (t w) -> p t w", w=W),
                            in0=simm[:].rearrange("p (t w) -> p t w", w=W),
                            in1=smax[:].unsqueeze(2).broadcast_to([P, T, W]),
                            op=OP.is_equal)
    e_all = persist.tile([P, T * W], f32)
    nc.scalar.copy(e_all[:], ninf_big[:])
    nc.vector.copy_predicated(e_all[:], eq[:], simm[:])

    # anti-diagonal scatter-max via PE shifted identities:
    # D_w[p, t] = E[128t + p - w] ; endv = max_w D_w.  Shift-by-w =
    # matmul with bigI slices (exact 0/1 weights; E uses -1e30 not -inf
    # so 0 * E stays 0).  Fake 0s only reach rows e < W < sep0+1, where
    # endv has no real contribution and end_logits is 0 either way.
    e3 = e_all[:].rearrange("p (t w) -> p t w", w=W)
    endv = persist.tile([P, T], f32)
    nc.vector.memset(endv[:], NEG)
    for w in range(W):
        psh = psh_p.tile([P, T], f32, tag="psh")
        nc.tensor.matmul(psh[:], bigI[:, W - w:W - w + P], e3[:, :, w],
                         start=True, stop=(w == 0))
        if w > 0:
            nc.tensor.matmul(psh[:, 1:T], bigI[:, W - w + P:W - w + 2 * P],
                             e3[:, 0:T - 1, w], start=False, stop=True)
        nc.vector.tensor_tensor(out=endv[:], in0=endv[:], in1=psh[:],
                                op=OP.max)

    # end_logits = where(endv == -inf, 0, endv)
    eq2 = persist.tile([P, T], mybir.dt.uint8)
    nc.vector.tensor_tensor(out=eq2[:], in0=endv[:], in1=ninf_big[:, 0:T],
                            op=OP.is_equal)
    end_lg = persist.tile([P, T], f32)
    nc.vector.select(end_lg[:], eq2[:], zeros16[:], endv[:])
    # start_logits = where(row_valid, smax, 0): invalid rows have smax=-1e30
    # and rvf=0, so a multiply gives (-)0.0 there.
    start_lg = persist.tile([P, T], f32)
    nc.vector.tensor_tensor(out=start_lg[:], in0=smax[:], in1=rvf[:],
                            op=OP.mult)

    # ---- phase D: stats + flip ----
    stat_row = persist.tile([1, P], f32)

    def cross_max(x16, out11, tagsfx):
        colmax = persist.tile([P, 1], f32, tag="colmax" + tagsfx)
        nc.vector.tensor_reduce(colmax[:], x16[:], axis=mybir.AxisListType.X,
                                op=OP.max)
        nc.sync.dma_start(stat_row[:], colmax[:])
        nc.vector.tensor_reduce(out11[:], stat_row[:],
                                axis=mybir.AxisListType.X, op=OP.max)

    def mean_std(x16, tagsfx):
        colsum = persist.tile([P, 1], f32, tag="cs" + tagsfx)
        nc.vector.tensor_reduce(colsum[:], x16[:], axis=mybir.AxisListType.X,
                                op=OP.add)
        ps = pst_p.tile([1, 1], f32, tag="ps_small")
        nc.tensor.matmul(ps[:], ones[:], colsum[:], start=True, stop=True)
        m = persist.tile([1, 1], f32, tag="m" + tagsfx)
        nc.scalar.mul(m[:], ps[:], 1.0 / S)
        negm = persist.tile([1, 1], f32, tag="nm" + tagsfx)
        nc.scalar.mul(negm[:], m[:], -1.0)
        negm_b = persist.tile([P, 1], f32, tag="nmb" + tagsfx)
        bcast_scalar(negm, negm_b, 1 if tagsfx == "s" else 2)
        scr = persist.tile([P, T], f32, tag="scr" + tagsfx)
        sqcol = persist.tile([P, 1], f32, tag="sq" + tagsfx)
        nc.scalar.activation(scr[:], x16[:], AF.Square, bias=negm_b[:],
                             accum_out=sqcol[:])
        ps2 = pst_p.tile([1, 1], f32, tag="ps_small")
        nc.tensor.matmul(ps2[:], ones[:], sqcol[:], start=True, stop=True)
        var = persist.tile([1, 1], f32, tag="v" + tagsfx)
        nc.scalar.mul(var[:], ps2[:], 1.0 / (S - 1))
        sd = persist.tile([1, 1], f32, tag="sd" + tagsfx)
        nc.scalar.activation(sd[:], var[:], AF.Sqrt)
        thr = persist.tile([1, 1], f32, tag="thr" + tagsfx)
        nc.vector.tensor_tensor(out=thr[:], in0=m[:], in1=sd[:], op=OP.add)
        return thr

    maxs = persist.tile([1, 1], f32)
    cross_max(start_lg, maxs, "s")
    thr_s = mean_std(start_lg, "s")
    thr_e = mean_std(end_lg, "e")
    fl_s = persist.tile([1, 1], mybir.dt.uint8)
    nc.vector.tensor_tensor(out=fl_s[:], in0=maxs[:], in1=thr_s[:], op=OP.is_lt)
    fl_e = persist.tile([1, 1], mybir.dt.uint8)
    nc.vector.tensor_tensor(out=fl_e[:], in0=maxs[:], in1=thr_e[:], op=OP.is_lt)
    flip = persist.tile([1, 1], mybir.dt.uint8)
    nc.vector.tensor_tensor(out=flip[:], in0=fl_s[:], in1=fl_e[:], op=OP.max)
    flip_b = persist.tile([P, 1], mybir.dt.uint8)
    nc.sync.dma_start(scb_d[0:1, 0:1], flip[:])
    nc.sync.dma_start(flip_b[:], scb_d[0:1, 0:1].broadcast_to([P, 1]))

    # ---- phase E: apply flip, write outputs ----
    for k, x16 in enumerate((start_lg, end_lg)):
        negx = persist.tile([P, T], f32, tag=f"negx{k}")
        nc.vector.tensor_scalar_mul(negx[:], x16[:], -1.0)
        isz = persist.tile([P, T], mybir.dt.uint8, tag=f"isz{k}")
        nc.vector.tensor_tensor(out=isz[:], in0=x16[:], in1=zeros16[:],
                                op=OP.is_equal)
        negged = persist.tile([P, T], f32, tag=f"ngd{k}")
        nc.vector.select(negged[:], isz[:], negm001[:], negx[:])
        outv = persist.tile([P, T], f32, tag=f"outv{k}")
        nc.vector.select(outv[:], flip_b[:].broadcast_to([P, T]), negged[:],
                         x16[:])
        nc.sync.dma_start(bass.AP(out_d.tensor, k * S, [[1, P], [P, T]]),
                          outv[:])


_NC_CACHE = {}


def build_program():
    key = 0
    if key in _NC_CACHE:
        return _NC_CACHE[key]
    nc = bacc.Bacc("TRN2", target_bir_lowering=False, debug=False)
    aps = {
        "seq": nc.dram_tensor("seq", [S, H], i16, kind="ExternalInput").ap(),
        "aux": nc.dram_tensor("aux", [P, AUXW], f32,
                              kind="ExternalInput").ap(),
        "out": nc.dram_tensor("out", [2, S], f32, kind="ExternalOutput").ap(),
        "qfd": nc.dram_tensor("qfd", [2 * H], f32).ap(),
        "d2f": nc.dram_tensor("d2f", [S + 32], f32).ap(),
        "nsf": nc.dram_tensor("nsf", [S + 32], f32).ap(),
        "sc": nc.dram_tensor("sc", [1, 8], f32).ap(),
        "scb": nc.dram_tensor("scb", [1, 8], mybir.dt.uint8).ap(),
    }
    with tile.TileContext(nc) as tc, ExitStack() as ctx:
        _emit(tc, ctx, aps)
    nc.compile()
    _NC_CACHE[key] = nc
    return nc


def host_prep(seq, idx):
    """Per-core derived inputs from one example. seq [S,H] f32, idx [2] int."""
    sep0, sep1 = int(idx[0]), int(idx[1])
    q1 = seq[1]
    q2 = seq[sep0 - 1]
    # Only rows [sep0+1, sep1) ever contribute (band mask kills the rest and
    # the q rows ship exactly in aux); zero-fill the others — the transport
    # layer compresses zero runs, so they cost ~nothing on the wire.
    used = slice(sep0 + 1, sep1)
    scale = np.float32(32767.0 / np.abs(seq[used]).max())
    xq = np.zeros((S, H), np.int16)
    xq[used] = np.rint(seq[used] * scale).astype(np.int16)
    aux = np.zeros((P, AUXW), np.float32)
    aux[:, 0:2 * C:2] = q1.reshape(C, P).T
    aux[:, 1:2 * C:2] = q2.reshape(C, P).T
    aux[:, 2 * C] = np.float32(sep0 + 1)
    aux[:, 2 * C + 1] = np.float32(sep1)
    return {"seq": xq, "aux": aux}


def kernel(sequence_outputs, idxs):
    sequence_outputs = np.asarray(sequence_outputs, dtype=np.float32)
    idxs = np.asarray(idxs)
    nc = build_program()
    in_maps = [host_prep(sequence_outputs[c], idxs[c]) for c in range(B)]
    res = run_bass_kernel_spmd(nc, in_maps, core_ids=list(range(B)))
    outs = np.stack([res.results[c]["out"] for c in range(B)])  # [B,2,S]
    start = np.ascontiguousarray(outs[:, 0, :])
    end = np.ascontiguousarray(outs[:, 1, :])
    return start, end
